# revision 3
# baseline (speedup 1.0000x reference)
"""Llama4-style MoE experts (grouped SwiGLU MLP) on Trainium2, 8 NeuronCores.

Expert-parallel: core i runs expert i's full MLP on its 1024-token slice:
    out = (up * silu(gate)) @ W2,  [gate|up] = h @ W1
Per-core shapes: h [1024, 2048], W1 [2048, 8192], W2 [4096, 2048].

Matmuls run in bf16 on the TensorEngine (1 cycle/row); f32 inputs are
cast on the Vector/Scalar engines which are otherwise mostly idle.
h is transposed on-chip via PE transpose-mode so both matmuls contract
over the partition dimension with natural DRAM layouts everywhere else.
"""

from contextlib import ExitStack

import numpy as np

import concourse.bass as bass
import concourse.mybir as mybir
import concourse.tile as tile
from concourse import bacc
from concourse.bass_utils import run_bass_kernel_spmd
from concourse.masks import make_identity

N_CORES = 8
P = 128
TB = 512  # moving-operand free-dim block (one PSUM bank of f32)

F32 = mybir.dt.float32
BF16 = mybir.dt.bfloat16
ACT_SIGMOID = mybir.ActivationFunctionType.Sigmoid

# Per-core problem dims (full problem: 8 experts x 1024 tokens, H=2048, F=4096)
T = 1024
H = 2048
F = 4096


def build_kernel_body(tc, T=T, H=H, F=F):
    nc = tc.nc
    h_d = nc.dram_tensor("hidden_states", [T, H], F32, kind="ExternalInput").ap()
    w1_d = nc.dram_tensor("gate_up_proj", [H, 2 * F], F32, kind="ExternalInput").ap()
    w2_d = nc.dram_tensor("down_proj", [F, H], F32, kind="ExternalInput").ap()
    out_d = nc.dram_tensor("out", [T, H], F32, kind="ExternalOutput").ap()

    n_ht = H // P          # h-tiles (contraction tiles of matmul 1)
    n_ft = F // P          # f-tiles (rows of act; contraction tiles of matmul 2)
    n_tt = T // P          # token tiles (psum partition tiles of matmul 2)
    n_tb = T // TB         # token free-dim blocks in matmul 1
    n_fb = F // TB         # 512-wide f blocks of W1 (per gate/up half)
    n_hb = H // TB         # 512-wide h blocks of W2

    with ExitStack() as ctx:
        const = ctx.enter_context(tc.tile_pool(name="const", bufs=1))
        hrow = ctx.enter_context(tc.tile_pool(name="hrow", bufs=2))
        htp = ctx.enter_context(tc.tile_pool(name="htp", bufs=n_ht))
        actp = ctx.enter_context(tc.tile_pool(name="actp", bufs=n_ft))
        wf = ctx.enter_context(tc.tile_pool(name="wf", bufs=8))
        w1b = ctx.enter_context(tc.tile_pool(name="w1b", bufs=80))
        w2b = ctx.enter_context(tc.tile_pool(name="w2b", bufs=8))
        silp = ctx.enter_context(tc.tile_pool(name="silp", bufs=6))
        outp = ctx.enter_context(tc.tile_pool(name="outp", bufs=6))
        ps = ctx.enter_context(tc.tile_pool(name="ps", bufs=8, space="PSUM"))

        ident = const.tile([P, P], F32, tag="ident", name="ident")
        make_identity(nc, ident)

        # ---- Phase A: hT[h, t] = h^T as bf16, 16 tiles of [128, T] ----
        ht = [
            htp.tile([P, T], BF16, tag="ht", name=f"ht{i}") for i in range(n_ht)
        ]
        for ti in range(n_tt):
            hr = hrow.tile([P, H], F32, tag="hrow", name=f"hrow{ti}")
            nc.sync.dma_start(hr[:], h_d[ti * P : (ti + 1) * P, :])
            for hh in range(n_ht):
                pt = ps.tile([P, TB], F32, tag="ps", name=f"tp{ti}_{hh}")
                nc.tensor.transpose(
                    pt[:, :P], hr[:, hh * P : (hh + 1) * P], ident
                )
                nc.any.tensor_copy(
                    out=ht[hh][:, ti * P : (ti + 1) * P], in_=pt[:, :P]
                )

        # ---- Phase B: G = h @ W1, act = up * silu(gate), stored [f, t] bf16 ----
        act = [
            actp.tile([P, T], BF16, tag="act", name=f"act{i}") for i in range(n_ft)
        ]
        for fb in range(n_fb):
            # Stream this f-block's W1 stripes (gate + up), cast to bf16
            # in [128, 128] tiles so consumption retires slots smoothly.
            wg = [[None] * 4 for _ in range(n_ht)]
            wu = [[None] * 4 for _ in range(n_ht)]
            for hh in range(n_ht):
                sg = wf.tile([P, TB], F32, tag="wf", name=f"w1g_{fb}_{hh}")
                nc.sync.dma_start(
                    sg[:],
                    w1_d[hh * P : (hh + 1) * P, fb * TB : (fb + 1) * TB],
                )
                su = wf.tile([P, TB], F32, tag="wf", name=f"w1u_{fb}_{hh}")
                nc.sync.dma_start(
                    su[:],
                    w1_d[hh * P : (hh + 1) * P, F + fb * TB : F + (fb + 1) * TB],
                )
                for i in range(4):
                    gb = w1b.tile([P, P], BF16, tag="w1b", name=f"w1gb_{fb}_{hh}_{i}")
                    nc.any.tensor_copy(out=gb[:], in_=sg[:, i * P : (i + 1) * P])
                    ub = w1b.tile([P, P], BF16, tag="w1b", name=f"w1ub_{fb}_{hh}_{i}")
                    nc.any.tensor_copy(out=ub[:], in_=su[:, i * P : (i + 1) * P])
                    wg[hh][i] = gb
                    wu[hh][i] = ub

            for i in range(4):
                fi = fb * 4 + i
                for tb in range(n_tb):
                    pg = ps.tile([P, TB], F32, tag="ps", name=f"pg{fi}_{tb}")
                    for hh in range(n_ht):
                        nc.tensor.matmul(
                            pg[:],
                            lhsT=wg[hh][i][:],
                            rhs=ht[hh][:, tb * TB : (tb + 1) * TB],
                            start=(hh == 0),
                            stop=(hh == n_ht - 1),
                        )
                    pu = ps.tile([P, TB], F32, tag="ps", name=f"pu{fi}_{tb}")
                    for hh in range(n_ht):
                        nc.tensor.matmul(
                            pu[:],
                            lhsT=wu[hh][i][:],
                            rhs=ht[hh][:, tb * TB : (tb + 1) * TB],
                            start=(hh == 0),
                            stop=(hh == n_ht - 1),
                        )
                    sig = silp.tile([P, TB], BF16, tag="silp", name=f"sig{fi}_{tb}")
                    nc.scalar.activation(sig[:], pg[:], ACT_SIGMOID)
                    tmp = silp.tile([P, TB], BF16, tag="tmpp", name=f"tmp{fi}_{tb}")
                    nc.vector.tensor_mul(out=tmp[:], in0=pu[:], in1=sig[:])
                    nc.vector.tensor_mul(
                        out=act[fi][:, tb * TB : (tb + 1) * TB],
                        in0=tmp[:],
                        in1=pg[:],
                    )

        # ---- Phase C: out = act @ W2, contracting over f ----
        for hb in range(n_hb):
            po = [
                ps.tile([P, TB], F32, tag="ps", name=f"po{hb}_{t}")
                for t in range(n_tt)
            ]
            for f in range(n_ft):
                s2 = wf.tile([P, TB], F32, tag="wf", name=f"w2f_{hb}_{f}")
                nc.sync.dma_start(
                    s2[:], w2_d[f * P : (f + 1) * P, hb * TB : (hb + 1) * TB]
                )
                b2 = w2b.tile([P, TB], BF16, tag="w2b", name=f"w2b_{hb}_{f}")
                nc.any.tensor_copy(out=b2[:], in_=s2[:])
                for tt in range(n_tt):
                    nc.tensor.matmul(
                        po[tt][:],
                        lhsT=act[f][:, tt * P : (tt + 1) * P],
                        rhs=b2[:],
                        start=(f == 0),
                        stop=(f == n_ft - 1),
                    )
            for tt in range(n_tt):
                ob = outp.tile([P, TB], F32, tag="outp", name=f"ob{hb}_{tt}")
                nc.any.tensor_copy(out=ob[:], in_=po[tt][:])
                nc.sync.dma_start(
                    out_d[tt * P : (tt + 1) * P, hb * TB : (hb + 1) * TB], ob[:]
                )


def build_nc(T=T, H=H, F=F):
    nc = bacc.Bacc(
        "TRN2", target_bir_lowering=False, debug=False, enable_asserts=False
    )
    with tile.TileContext(nc) as tc:
        build_kernel_body(tc, T=T, H=H, F=F)
    nc.compile()
    return nc


_NC_CACHE = None


def run(hidden_states, gate_up_proj, down_proj, trace=False, **kw):
    """Run on the 8 NeuronCores; returns (output, BassKernelResults)."""
    global _NC_CACHE
    if _NC_CACHE is None:
        _NC_CACHE = build_nc()
    nc = _NC_CACHE

    hs = np.ascontiguousarray(np.asarray(hidden_states), dtype=np.float32)
    gup = np.ascontiguousarray(np.asarray(gate_up_proj), dtype=np.float32)
    dp = np.ascontiguousarray(np.asarray(down_proj), dtype=np.float32)
    assert hs.shape == (N_CORES * T, H), hs.shape
    assert gup.shape == (N_CORES, H, 2 * F), gup.shape
    assert dp.shape == (N_CORES, F, H), dp.shape

    in_maps = [
        {
            "hidden_states": np.ascontiguousarray(hs[i * T : (i + 1) * T]),
            "gate_up_proj": np.ascontiguousarray(gup[i]),
            "down_proj": np.ascontiguousarray(dp[i]),
        }
        for i in range(N_CORES)
    ]
    res = run_bass_kernel_spmd(
        nc, in_maps, core_ids=list(range(N_CORES)), trace=trace, **kw
    )
    out = np.concatenate(
        [res.results[i]["out"] for i in range(N_CORES)], axis=0
    ).astype(np.float32)
    return out, res


def kernel(hidden_states, gate_up_proj, down_proj):
    out, _ = run(hidden_states, gate_up_proj, down_proj, trace=False)
    return out
